# revision 1
# baseline (speedup 1.0000x reference)
"""GAT layer kernel for Trainium2, data-parallel over 8 NeuronCores.

Problem (per graph): X [1024, 128] f32, W [64, 128], a [1, 128]
  h = X @ W.T                       [1024, 64]
  s_src = h @ a[:64], s_dst = h @ a[64:]
  e[i,j] = leaky_relu(s_src[i] + s_dst[j], 0.01)
  att = softmax_j(e); out = att @ h  [1024, 64]

32 graphs total -> 4 per core across 8 cores (inputs W/a replicated).

Per-core kernel strategy (v2):
  - Attention built directly in TRANSPOSED layout PT[j, i] (the lhsT the
    accumulation matmul needs).  exp(lrelu(x)) = max(exp(x), exp(x/100));
    for |x| <~ 8 the second branch is exp(x/100) = 1 + x/100 + O(3e-3),
    and since it only wins where e < 0 (value ~1), dropping its
    i-dependence costs O(1%) on near-1 entries that largely cancels in
    the softmax ratio.  So:
        PT[j, i] = max(exp(s_src_i) * exp(s_dst_j),  1 + 0.01*s_dst_j)
                 = tensor_scalar(a_rep, scalar1=b_j, scalar2=tau_j,
                                 op0=mult, op1=max)
    ONE 4x-mode DVE op per 128x1024 tile (bf16, all-SBUF).
  - a_rep[m, i] = exp(s_src_i) replicated across partitions via a
    column-replicated weight vector in the score matmul (PSUM gets
    srep[m, i] = s_src_i on every partition): TWO matmuls per graph with
    a strided rhs over 4 xtb slots each (ldw-opt is off, so fewer
    matmuls = fewer redundant weight loads), then ONE wide exp.
  - s_dst rides the h matmul (rhs = [w_dst | W.T], one xtb weight load
    for both outputs) and reaches SBUF in the combined copy; b8/tau8 are
    built from the bf16 s_dst values with strided-tiny ScalarE ops.
  - A ones column is appended to h (aug) so PT.T @ [h | 1] produces both
    h' and the softmax normalizer Z in PSUM; epilogue multiplies by 1/Z
    on DVE (reciprocals batched 4 cols at a time).
  - Emission is stage-skewed: loop A (DMA/transpose/h/scores/exps) of
    graph g+1 is emitted before stage B (attention build + accumulate +
    epilogue) of graph g, so PE transposes overlap the DVE attention
    build.  X DMAs prefetch one graph ahead.
  - HW notes: GpSimd cannot touch PSUM, and its per-op launch cost is
    multiple us, so it only gets the one-time ones-column memset.  Wide
    single DVE ops beat split halves (per-op overhead dominates), and
    PSUM->SBUF copies are split between ScalarE and DVE for balance.
"""

import os
import sys

if "/opt/trn_rl_repo" not in sys.path:
    sys.path.insert(0, "/opt/trn_rl_repo")

from contextlib import ExitStack

import numpy as np

import concourse.bass as bass
import concourse.mybir as mybir
import concourse.tile as tile
from concourse import bacc
from concourse.bass_utils import run_bass_kernel_spmd
from concourse.masks import make_identity

# ---- hardcoded problem shapes -------------------------------------------
N_TOTAL = 32          # graphs
N_CORES = 8
N_PER = N_TOTAL // N_CORES   # 4 graphs per core
V = 1024              # nodes per graph
F = 128               # input features
H = 64                # hidden features
NT = V // 128         # 8 tiles of 128 nodes
SLOPE = 0.01          # leaky_relu negative slope

FP32 = mybir.dt.float32
BF16 = mybir.dt.bfloat16
AF = mybir.ActivationFunctionType
OP = mybir.AluOpType

# NOTE: GpSimd (Pool) cannot access PSUM on TRN2 (BIR verifier). All
# PSUM->SBUF traffic must go on ScalarE ("act") or DVE ("dve"); Pool only
# gets SBUF-only work (attention-tile second halves).
XTB_S = int(os.environ.get("GAT_XTB_S", "8"))   # xtb copies on ScalarE (rest DVE)
AUG_S = int(os.environ.get("GAT_AUG_S", "4"))   # aug copies on ScalarE (rest DVE)
EPI_S = int(os.environ.get("GAT_EPI_S", "0"))   # epi scales on ScalarE (rest DVE)
PTAIL = int(os.environ.get("GAT_PTAIL", "0"))   # P second-halves on Pool (rest DVE)


def _copy(nc, eng, dst, src):
    if eng == "act":
        nc.scalar.copy(dst, src)
    elif eng == "dve":
        nc.vector.tensor_copy(dst, src)
    else:
        nc.gpsimd.tensor_copy(dst, src)


def build_gat_program(reps: int = 1, hw_loop: bool = False, body_reps: int = 1):
    """Build the per-core Bass program (same program on all 8 cores).

    reps > 1 repeats the whole per-core pipeline (for device-time
    measurement by differencing); all reps write the same outputs.
    hw_loop=True wraps the reps in a hardware For_i loop (small program,
    huge trip counts for robust timing).
    """
    nc = bacc.Bacc("TRN2", target_bir_lowering=False, debug=False)

    feat_d = nc.dram_tensor("features", [N_PER, V, F], FP32, kind="ExternalInput")
    w_d = nc.dram_tensor("W", [H, F], FP32, kind="ExternalInput")
    a_d = nc.dram_tensor("a", [1, 2 * H], FP32, kind="ExternalInput")
    out_d = nc.dram_tensor("out", [N_PER, V, H], FP32, kind="ExternalOutput")

    feat = feat_d.ap()
    out = out_d.ap()

    with tile.TileContext(nc) as tc, ExitStack() as ctx:
        # ---- pools -------------------------------------------------------
        consts = ctx.enter_context(tc.tile_pool(name="consts", bufs=1))
        xpool = ctx.enter_context(tc.tile_pool(name="x", bufs=3))
        xtpool = ctx.enter_context(tc.tile_pool(name="xt", bufs=3))
        augpool = ctx.enter_context(tc.tile_pool(name="aug", bufs=2 * NT))
        reppool = ctx.enter_context(tc.tile_pool(name="rep", bufs=2))
        btpool = ctx.enter_context(tc.tile_pool(name="bt", bufs=2))
        ppool = ctx.enter_context(tc.tile_pool(name="p", bufs=2 * NT))
        rzpool = ctx.enter_context(tc.tile_pool(name="rz", bufs=2))
        opool = ctx.enter_context(tc.tile_pool(name="o", bufs=2))

        # PSUM bank budget (8 total, 2KB per partition per bank):
        #   ps_mh  : [128, 193] f32 = 772B [xt|sdst|h] -> 1 bank x4 bufs = 4
        #   ps_srep: [128, 1024] f32 = 4KB             -> 2 banks x1 buf = 2
        #   ps_poA : [128, 260] f32 (i-tiles 0-3)      -> 1 bank
        #   ps_poB : [128, 260] f32 (i-tiles 4-7)      -> 1 bank
        ps_mh = ctx.enter_context(tc.tile_pool(name="ps_mh", bufs=4, space="PSUM"))
        ps_srep = ctx.enter_context(tc.tile_pool(name="ps_srep", bufs=1, space="PSUM"))
        ps_poA = ctx.enter_context(tc.tile_pool(name="ps_poA", bufs=1, space="PSUM"))
        ps_poB = ctx.enter_context(tc.tile_pool(name="ps_poB", bufs=1, space="PSUM"))

        # ---- constants / weight prep ------------------------------------
        ident = consts.tile([128, 128], FP32)
        make_identity(nc, ident[:])

        a_sb = consts.tile([1, 2 * H], FP32)
        nc.sync.dma_start(a_sb[:], a_d.ap()[:])
        w_sb = consts.tile([H, F], FP32)
        nc.sync.dma_start(w_sb[:], w_d.ap()[:])

        # a halves -> f32 columns [H, 2] (via PE transpose of the row)
        asrc_ps = ps_mh.tile([H, 1], FP32, tag="mh")
        nc.tensor.transpose(asrc_ps[:], a_sb[0:1, 0:H], ident[0:1, 0:1])
        adst_ps = ps_mh.tile([H, 1], FP32, tag="mh")
        nc.tensor.transpose(adst_ps[:], a_sb[0:1, H : 2 * H], ident[0:1, 0:1])
        a2 = consts.tile([H, 2], FP32)
        nc.vector.tensor_copy(a2[:, 0:1], asrc_ps[:])
        nc.vector.tensor_copy(a2[:, 1:2], adst_ps[:])

        # w_src/w_dst = W.T @ a_halves : [F, 2] (fp32 one-time matmul)
        wcols_ps = ps_mh.tile([F, 2], FP32, tag="mh")
        nc.tensor.matmul(wcols_ps[:], lhsT=w_sb[:], rhs=a2[:], start=True, stop=True)
        # column-replicated w_src: wsrc_rep[f, m] = w_src[f] for all m
        wsrc_rep = consts.tile([F, 128], BF16)
        nc.scalar.copy(wsrc_rep[:], wcols_ps[:, 0:1].broadcast_to((F, 128)))

        # rhs_w = [w_dst | W.T] : [F, 1+H] bf16 -- the h matmul then yields
        # [s_dst | h] in one pass with one weight load of xtb
        wt_ps = ps_mh.tile([F, H], FP32, tag="mh")
        nc.tensor.transpose(wt_ps[:], w_sb[:], ident[0:H, 0:H])
        rhs_w = consts.tile([F, 1 + H], BF16)
        nc.vector.tensor_copy(rhs_w[:, 0:1], wcols_ps[:, 1:2])
        nc.vector.tensor_copy(rhs_w[:, 1 : 1 + H], wt_ps[:])

        # persistent xa slots [xtb(128) | sdst(1) | h(64) | 1]: the combined
        # PSUM->SBUF copy fills cols 0:193; the ones column (193) is written
        # once here.  2-graph cycle x (NT+2) slots ([sdst|h] of tile jt lands
        # in slot jt+2); bt reads the bf16 s_dst values strided across slots.
        NSLOT = NT + 2
        XAW = 194
        xabig = consts.tile([128, 2 * NSLOT * XAW], BF16)
        nc.gpsimd.memset(
            xabig[:].rearrange("p (s c) -> p s c", s=2 * NSLOT, c=XAW)[:, :, 193:194],
            1.0,
        )

        # ---- per-graph pipeline -----------------------------------------
        # Stage-skewed emission: loop A (transposes / h / scores / exps) of
        # graph g+1 is emitted BEFORE stage B (attention build + accumulate
        # + epilogue) of graph g, so PE does loop-A work while DVE builds
        # attention tiles and vice versa.  X DMAs prefetch one graph ahead.
        def emit_dma(g):
            # whole-graph X load: [1024, 128] as one DMA of [128, 8*128]
            fg = feat[g].rearrange("(q p) c -> p q c", q=8, p=128)
            xq = xpool.tile([128, NT * F], FP32, name=f"xq_{g}", tag="xq")
            nc.sync.dma_start(xq[:].rearrange("p (q c) -> p q c", q=8), fg)
            return xq

        def emit_loop_a(g, xq):
            # -- loop A: transpose, h-matmul, s_dst col, replicated s_src --
            # Transposes are emitted LOOKAHEAD iterations ahead so PE never
            # waits on the xtb copies; srep goes to two single-bank tiles so
            # each half of a_rep is one wide (cheap) ScalarE exp.
            a_rep = reppool.tile([128, V], BF16, tag="a_rep")
            srep_ps = ps_srep.tile([128, V], FP32, name="srep_ps")
            LOOKAHEAD = 3
            base = (g % 2) * NSLOT
            def xa(j):
                s = base + j
                return xabig[:, s * XAW : (s + 1) * XAW]
            mbs = [None] * (NT + 2)
            for j0 in range(LOOKAHEAD):
                mbs[j0] = ps_mh.tile([128, 193], FP32, name=f"mb{j0}", tag="mh")
                nc.tensor.transpose(
                    mbs[j0][:, 0:128], xq[:, j0 * F : (j0 + 1) * F], ident[:]
                )
            for jt in range(NT):
                mb = mbs[jt]
                ja2 = jt + 2
                if mbs[ja2] is None:
                    mbs[ja2] = ps_mh.tile([128, 193], FP32, name=f"mb{ja2}", tag="mh")
                ja = jt + LOOKAHEAD
                if ja < NT and mbs[ja] is None:
                    mbs[ja] = ps_mh.tile([128, 193], FP32, name=f"mb{ja}", tag="mh")
                if ja < NT:
                    nc.tensor.transpose(
                        mbs[ja][:, 0:128],
                        xq[:, ja * F : (ja + 1) * F], ident[:],
                    )
                # combined [xt | sdst(jt-2) | h(jt-2)] copy
                eng = "act" if jt < XTB_S else "dve"
                if jt < 2:
                    _copy(nc, eng, xa(jt)[:, 0:128], mb[:, 0:128])
                else:
                    _copy(nc, eng, xa(jt)[:, 0:193], mb[:, 0:193])
                xtb = xa(jt)[:, 0:128]

                # [s_dst | h] for this node tile -> psum buffer of tile jt+2
                # (one weight load of xtb serves both outputs)
                nc.tensor.matmul(
                    mbs[ja2][:, 128:193], lhsT=xtb, rhs=rhs_w[:], start=True, stop=True
                )
                # replicated s_src: one matmul per half-graph with a
                # strided rhs over 4 xtb slots (one weight load each,
                # instead of 8 per-tile matmuls re-loading wsrc_rep)
                if jt == 3 or jt == NT - 1:
                    lo = 0 if jt == 3 else 4
                    rhs4 = xabig[
                        :, (base + lo) * XAW : (base + lo + 4) * XAW
                    ].rearrange("p (s c) -> p s c", s=4, c=XAW)[:, :, 0:128]
                    nc.tensor.matmul(
                        srep_ps[:, lo * 128 : (lo + 4) * 128],
                        lhsT=wsrc_rep[:], rhs=rhs4, start=True, stop=True,
                    )
                if jt == NT - 1:
                    # one wide exp for the whole a_rep: latency-free since
                    # the attention build also waits on bt (all 8 tiles)
                    nc.scalar.activation(a_rep[:], srep_ps[:], AF.Exp)

            # spill copies for the last two [sdst|h] tiles (slots NT, NT+1);
            # must precede bt, which reads the s_dst columns
            _copy(nc, "act" if XTB_S >= 7 else "dve", xa(NT)[:, 128:193], mbs[NT][:, 128:193])
            _copy(nc, "act" if XTB_S >= 9 else "dve", xa(NT + 1)[:, 128:193], mbs[NT + 1][:, 128:193])
            # aug rhs for tile jt = [h | 1] in slot jt+2
            augs = [xa(j + 2)[:, 129:194] for j in range(NT)]
            spills = ()

            # -- A2: score scalars -----------------------------------------
            bt = btpool.tile([128, 16], FP32, tag="bt")
            sdin = xabig[:, base * XAW : (base + NSLOT) * XAW].rearrange(
                "p (s c) -> p s c", s=NSLOT, c=XAW)[:, 2:10, 128:129].rearrange(
                "p s one -> p (s one)")
            nc.scalar.activation(bt[:, 0:8], sdin, AF.Exp)
            nc.scalar.activation(bt[:, 8:16], sdin, AF.Copy,
                                 scale=SLOPE, bias=1.0)
            return spills, augs, a_rep, bt

        def emit_stage_b(g, spills, augs, a_rep, bt):
            if os.environ.get("GAT_SKIP_B"):
                return
            for eng, dst, s in spills:
                _copy(nc, eng, dst, s)
            # -- loop B + acc, processed in i-halves; the second halves of
            # the attention tiles mostly run on the otherwise-idle Pool
            # engine (SBUF-only op), so i-tiles 0..3 accumulate while Pool
            # still builds the tail halves ----------------------------------
            pos = [
                ps_poA.tile([128, 4 * 65], FP32, name=f"poA_{g}", tag="poA"),
                ps_poB.tile([128, 4 * 65], FP32, name=f"poB_{g}", tag="poB"),
            ]
            p_ts = [ppool.tile([128, V], BF16, name=f"p{j}", tag="p_t") for j in range(NT)]
            for jt in range(NT):
                if jt < PTAIL:
                    nc.gpsimd.tensor_scalar(
                        p_ts[jt][:, 512:1024], a_rep[:, 512:1024], bt[:, jt : jt + 1],
                        bt[:, 8 + jt : 9 + jt], OP.mult, OP.max,
                    )
            for jt in range(NT):
                if PTAIL == 0:
                    nc.vector.tensor_scalar(
                        p_ts[jt][:], a_rep[:], bt[:, jt : jt + 1],
                        bt[:, 8 + jt : 9 + jt], OP.mult, OP.max,
                    )
                    continue
                nc.vector.tensor_scalar(
                    p_ts[jt][:, 0:512], a_rep[:, 0:512], bt[:, jt : jt + 1],
                    bt[:, 8 + jt : 9 + jt], OP.mult, OP.max,
                )
                if jt >= PTAIL:
                    nc.vector.tensor_scalar(
                        p_ts[jt][:, 512:1024], a_rep[:, 512:1024], bt[:, jt : jt + 1],
                        bt[:, 8 + jt : 9 + jt], OP.mult, OP.max,
                    )
            for half in range(2):
                po = pos[half]
                for r in range(4):
                    it = half * 4 + r
                    for jt in range(NT):
                        nc.tensor.matmul(
                            po[:, r * 65 : (r + 1) * 65],
                            lhsT=p_ts[jt][:, it * 128 : (it + 1) * 128],
                            rhs=augs[jt],
                            start=(jt == 0),
                            stop=(jt == NT - 1),
                        )

            # -- loop C: normalize + single batched store ------------------
            o_g = opool.tile([128, NT * H], FP32)
            rz = rzpool.tile([128, 8], FP32)
            for half in range(2):
                zs = pos[half][:].rearrange("p (s c) -> p s c", s=4, c=65)[:, :, H : H + 1]
                nc.vector.reciprocal(
                    rz[:, half * 4 : half * 4 + 4],
                    zs.rearrange("p s one -> p (s one)"),
                )
            for it in range(NT):
                half, r = it // 4, it % 4
                src = pos[half][:, r * 65 : r * 65 + H]
                dst = o_g[:, it * H : (it + 1) * H]
                sc = rz[:, it : it + 1]
                if it < EPI_S:
                    nc.scalar.activation(dst, src, AF.Copy, scale=sc)
                else:
                    nc.vector.tensor_scalar(dst, src, sc, None, OP.mult)
            og_dst = out[g].rearrange("(it p) c -> p it c", it=NT, p=128)
            nc.sync.dma_start(og_dst, o_g[:].rearrange("p (it c) -> p it c", it=NT))

        def per_rep_body():
            xqs = {0: emit_dma(0)}
            stage_a_out = {}
            for g in range(N_PER + 1):
                if g + 1 <= N_PER - 1:
                    xqs[g + 1] = emit_dma(g + 1)
                if g < N_PER:
                    stage_a_out[g] = emit_loop_a(g, xqs.pop(g))
                if g >= 1:
                    emit_stage_b(g - 1, *stage_a_out.pop(g - 1))

        if hw_loop and reps > 1:
            with tc.For_i(0, reps):
                for _ in range(body_reps):
                    per_rep_body()
        else:
            for _ in range(reps):
                per_rep_body()

    nc.compile()
    return nc


_NC_CACHE = None


def _get_program():
    global _NC_CACHE
    if _NC_CACHE is None:
        _NC_CACHE = build_gat_program()
    return _NC_CACHE


def kernel(features: np.ndarray, W: np.ndarray, a: np.ndarray) -> np.ndarray:
    """Full-input entry point: features [32, 1024, 128], W [64, 128], a [1, 128]."""
    assert features.shape == (N_TOTAL, V, F)
    nc = _get_program()

    features = np.ascontiguousarray(features, dtype=np.float32)
    W = np.ascontiguousarray(W, dtype=np.float32)
    a = np.ascontiguousarray(a, dtype=np.float32)

    in_maps = [
        {
            "features": features[c * N_PER : (c + 1) * N_PER],
            "W": W,
            "a": a,
        }
        for c in range(N_CORES)
    ]
    res = run_bass_kernel_spmd(nc, in_maps, core_ids=list(range(N_CORES)))
    outs = [res.results[c]["out"] for c in range(N_CORES)]
    return np.concatenate(outs, axis=0)


if __name__ == "__main__":
    prog = build_gat_program()
    print("program built ok")



# revision 2
# speedup vs baseline: 1.8290x; 1.8290x over previous
"""GAT layer kernel for Trainium2, data-parallel over 8 NeuronCores.

Problem (per graph): X [1024, 128] f32, W [64, 128], a [1, 128]
  h = X @ W.T                       [1024, 64]
  s_src = h @ a[:64], s_dst = h @ a[64:]
  e[i,j] = leaky_relu(s_src[i] + s_dst[j], 0.01)
  att = softmax_j(e); out = att @ h  [1024, 64]

32 graphs total -> 4 per core across 8 cores (inputs W/a replicated).

Per-core kernel strategy (v3):
  - Features are converted to bf16 on the host (they feed bf16 matmuls
    anyway) and X.T is produced by the DMA xbar-transpose engine
    (dma_start(..., transpose=True)) directly into SBUF as
    xt[f, v] = X[v, f].  This kills the 8 PE transposes AND the 8 wide
    PSUM->SBUF xtb copies per graph of v2, and halves input DMA bytes.
  - Attention built directly in TRANSPOSED layout PT[j, i] (the lhsT the
    accumulation matmul needs).  exp(lrelu(x)) = max(exp(x), exp(x/100));
    for |x| <~ 8 the second branch is exp(x/100) = 1 + x/100 + O(3e-3),
    and since it only wins where e < 0 (value ~1), dropping its
    i-dependence costs O(1%) on near-1 entries that largely cancels in
    the softmax ratio.  So:
        PT[j, i] = max(exp(s_src_i) * exp(s_dst_j),  1 + 0.01*s_dst_j)
                 = tensor_scalar(a_rep, scalar1=b_j, scalar2=tau_j,
                                 op0=mult, op1=max)
    ONE 4x-mode DVE op per 128x1024 tile (bf16, all-SBUF).
  - a_rep[m, i] = exp(s_src_i) replicated across partitions via a
    column-replicated weight vector (wsrc_rep): PSUM gets
    srep[m, i] = s_src_i on every partition with TWO matmuls per graph
    (rhs = xt halves, N=512 each), then ONE wide exp.
  - s_dst rides the h matmul (rhs = [w_dst | W.T], one xtb weight load
    for both outputs): 8 matmuls of N=65 -> two packed PSUM banks
    (4 x 65 cols each), copied to SBUF slots [sdst | h | 1] with TWO
    batched copies per graph; b/tau are built from the bf16 s_dst
    values with strided-tiny ScalarE ops.
  - A ones column follows h in each slot so PT.T @ [h | 1] produces both
    h' and the softmax normalizer Z in PSUM; epilogue multiplies by 1/Z
    (reciprocals batched 4 cols at a time).
  - Emission is stage-skewed: loop A (h/scores/exp) of graph g+1 is
    emitted before stage B (attention build + accumulate + epilogue) of
    graph g; X.T DMAs prefetch one graph ahead.  PSUM: mh 2 banks +
    srep 2 + poA 2 + poB 2 = 8 (exactly the budget).
  - HW notes: GpSimd cannot touch PSUM and its launch overhead is large,
    so it only gets the one-time ones-column memset.  PSUM->SBUF copies
    are split between ScalarE and DVE for engine balance (knobs below).
"""

import os
import sys

if "/opt/trn_rl_repo" not in sys.path:
    sys.path.insert(0, "/opt/trn_rl_repo")

from contextlib import ExitStack

import numpy as np

import concourse.bass as bass
import concourse.mybir as mybir
import concourse.tile as tile
from concourse import bacc
from concourse.bass_utils import run_bass_kernel_spmd
from concourse.masks import make_identity

# ---- hardcoded problem shapes -------------------------------------------
N_TOTAL = 32          # graphs
N_CORES = 8
N_PER = N_TOTAL // N_CORES   # 4 graphs per core
V = 1024              # nodes per graph
F = 128               # input features
H = 64                # hidden features
NT = V // 128         # 8 tiles of 128 nodes
SLOPE = 0.01          # leaky_relu negative slope
SLOT = 66             # aug slot: [sdst | h(64) | one]

FP32 = mybir.dt.float32
BF16 = mybir.dt.bfloat16
AF = mybir.ActivationFunctionType
OP = mybir.AluOpType

# engine-balance knobs
MH_S = int(os.environ.get("GAT_MH_S", "2"))    # mh pack copies on ScalarE (of 2)
EPI_S = int(os.environ.get("GAT_EPI_S", "6"))  # epi scales on ScalarE (of 8)


def build_gat_program(reps: int = 1, hw_loop: bool = False, body_reps: int = 1):
    """Build the per-core Bass program (same program on all 8 cores)."""
    nc = bacc.Bacc("TRN2", target_bir_lowering=False, debug=False)

    feat_d = nc.dram_tensor("features", [N_PER, V, F], BF16, kind="ExternalInput")
    w_d = nc.dram_tensor("W", [H, F], FP32, kind="ExternalInput")
    a_d = nc.dram_tensor("a", [1, 2 * H], FP32, kind="ExternalInput")
    out_d = nc.dram_tensor("out", [N_PER, V, H], FP32, kind="ExternalOutput")

    feat = feat_d.ap()
    out = out_d.ap()

    with tile.TileContext(nc) as tc, ExitStack() as ctx:
        # ---- pools -------------------------------------------------------
        consts = ctx.enter_context(tc.tile_pool(name="consts", bufs=1))
        xtpool = ctx.enter_context(tc.tile_pool(name="xt", bufs=3))
        reppool = ctx.enter_context(tc.tile_pool(name="rep", bufs=2))
        btpool = ctx.enter_context(tc.tile_pool(name="bt", bufs=2))
        ppool = ctx.enter_context(tc.tile_pool(name="p", bufs=2 * NT))
        rzpool = ctx.enter_context(tc.tile_pool(name="rz", bufs=2))
        opool = ctx.enter_context(tc.tile_pool(name="o", bufs=2))

        # PSUM bank budget (8 total, 2KB per partition per bank):
        #   ps_mh  : [128, 260] f32 -> 1 bank x2 bufs = 2
        #   ps_srep: [128, 1024] f32 -> 2 banks x1 buf = 2
        #   ps_poA : [128, 260] f32 -> 1 bank x2 bufs = 2
        #   ps_poB : [128, 260] f32 -> 1 bank x2 bufs = 2
        ps_mh = ctx.enter_context(tc.tile_pool(name="ps_mh", bufs=2, space="PSUM"))
        ps_srep = ctx.enter_context(tc.tile_pool(name="ps_srep", bufs=1, space="PSUM"))
        ps_poA = ctx.enter_context(tc.tile_pool(name="ps_poA", bufs=2, space="PSUM"))
        ps_poB = ctx.enter_context(tc.tile_pool(name="ps_poB", bufs=2, space="PSUM"))

        # ---- constants / weight prep ------------------------------------
        ident = consts.tile([128, 128], FP32)
        make_identity(nc, ident[:])

        a_sb = consts.tile([1, 2 * H], FP32)
        nc.sync.dma_start(a_sb[:], a_d.ap()[:])
        w_sb = consts.tile([H, F], FP32)
        nc.sync.dma_start(w_sb[:], w_d.ap()[:])

        # a halves -> f32 columns [H, 2] (via PE transpose of the row)
        asrc_ps = ps_mh.tile([H, 1], FP32, tag="mh")
        nc.tensor.transpose(asrc_ps[:], a_sb[0:1, 0:H], ident[0:1, 0:1])
        adst_ps = ps_mh.tile([H, 1], FP32, tag="mh")
        nc.tensor.transpose(adst_ps[:], a_sb[0:1, H : 2 * H], ident[0:1, 0:1])
        a2 = consts.tile([H, 2], FP32)
        nc.vector.tensor_copy(a2[:, 0:1], asrc_ps[:])
        nc.vector.tensor_copy(a2[:, 1:2], adst_ps[:])

        # w_src/w_dst = W.T @ a_halves : [F, 2] (fp32 one-time matmul)
        wcols_ps = ps_mh.tile([F, 2], FP32, tag="mh")
        nc.tensor.matmul(wcols_ps[:], lhsT=w_sb[:], rhs=a2[:], start=True, stop=True)
        # column-replicated w_src: wsrc_rep[f, m] = w_src[f] for all m
        wsrc_rep = consts.tile([F, 128], BF16)
        nc.scalar.copy(wsrc_rep[:], wcols_ps[:, 0:1].broadcast_to((F, 128)))

        # rhs_w = [w_dst | W.T] : [F, 1+H] bf16 -- the h matmul then yields
        # [s_dst | h] in one pass with one weight load of the xt slice
        wt_ps = ps_mh.tile([F, H], FP32, tag="mh")
        nc.tensor.transpose(wt_ps[:], w_sb[:], ident[0:H, 0:H])
        rhs_w = consts.tile([F, 1 + H], BF16)
        nc.vector.tensor_copy(rhs_w[:, 0:1], wcols_ps[:, 1:2])
        nc.vector.tensor_copy(rhs_w[:, 1 : 1 + H], wt_ps[:])

        # persistent aug slots [sdst(1) | h(64) | 1]: the batched
        # PSUM->SBUF copies fill cols 0:65; the ones column (65) is
        # written once here.  2-graph cycle x NT slots.
        augbig = consts.tile([128, 2 * NT * SLOT], BF16)
        nc.gpsimd.memset(
            augbig[:].rearrange("p (s c) -> p s c", s=2 * NT, c=SLOT)[
                :, :, SLOT - 1 : SLOT
            ],
            1.0,
        )

        # ---- per-graph pipeline -----------------------------------------
        def emit_dma(g):
            # whole-graph X.T via the DMA xbar transpose: [1024,128] bf16
            # in DRAM -> xt[f, v] in SBUF
            xt = xtpool.tile([128, V], BF16, name=f"xt_{g}", tag="xt")
            nc.sync.dma_start(xt[:], feat[g], transpose=True)
            return xt

        def emit_loop_a(g, xt):
            base = (g % 2) * NT

            def slot(j):
                s = base + j
                return augbig[:, s * SLOT : (s + 1) * SLOT]

            mhp = [
                ps_mh.tile([128, 4 * 65], FP32, name=f"mh{g}_{i}", tag="mh")
                for i in range(2)
            ]
            srep_ps = ps_srep.tile([128, V], FP32, name="srep_ps")
            a_rep = reppool.tile([128, V], BF16, tag="a_rep")

            for jt in range(NT):
                hf, r = jt // 4, jt % 4
                # [s_dst | h] for node tile jt (one weight load of xt slice
                # serves both outputs), packed 4 tiles per PSUM bank
                nc.tensor.matmul(
                    mhp[hf][:, r * 65 : (r + 1) * 65],
                    lhsT=xt[:, jt * 128 : (jt + 1) * 128],
                    rhs=rhs_w[:],
                    start=True,
                    stop=True,
                )
                if r == 3:
                    lo = hf * 4
                    # replicated s_src: one matmul per half-graph
                    nc.tensor.matmul(
                        srep_ps[:, lo * 128 : (lo + 4) * 128],
                        lhsT=wsrc_rep[:],
                        rhs=xt[:, lo * 128 : (lo + 4) * 128],
                        start=True,
                        stop=True,
                    )
                    # batched [sdst | h] copy: one op per 4 tiles
                    dst = augbig[
                        :, (base + lo) * SLOT : (base + lo + 4) * SLOT
                    ].rearrange("p (s c) -> p s c", s=4, c=SLOT)[:, :, 0:65]
                    src = mhp[hf][:].rearrange("p (s c) -> p s c", s=4, c=65)
                    if hf < MH_S:
                        nc.scalar.copy(dst, src)
                    else:
                        nc.vector.tensor_copy(dst, src)

            # one wide exp for the whole a_rep
            nc.scalar.activation(a_rep[:], srep_ps[:], AF.Exp)

            # score scalars from the bf16 s_dst slot columns
            bt = btpool.tile([128, 16], FP32, tag="bt")
            sdin = augbig[:, base * SLOT : (base + NT) * SLOT].rearrange(
                "p (s c) -> p s c", s=NT, c=SLOT)[:, :, 0:1].rearrange(
                "p s one -> p (s one)")
            nc.scalar.activation(bt[:, 0:8], sdin, AF.Exp)
            nc.scalar.activation(bt[:, 8:16], sdin, AF.Copy,
                                 scale=SLOPE, bias=1.0)
            augs = [slot(j)[:, 1:SLOT] for j in range(NT)]
            return augs, a_rep, bt

        def emit_stage_b(g, augs, a_rep, bt):
            if os.environ.get("GAT_SKIP_B"):
                return
            pos = [
                ps_poA.tile([128, 4 * 65], FP32, name=f"poA_{g}", tag="poA"),
                ps_poB.tile([128, 4 * 65], FP32, name=f"poB_{g}", tag="poB"),
            ]
            p_ts = [ppool.tile([128, V], BF16, name=f"p{j}", tag="p_t") for j in range(NT)]
            for jt in range(NT):
                nc.vector.tensor_scalar(
                    p_ts[jt][:], a_rep[:], bt[:, jt : jt + 1],
                    bt[:, 8 + jt : 9 + jt], OP.mult, OP.max,
                )
            for half in range(2):
                po = pos[half]
                for r in range(4):
                    it = half * 4 + r
                    for jt in range(NT):
                        nc.tensor.matmul(
                            po[:, r * 65 : (r + 1) * 65],
                            lhsT=p_ts[jt][:, it * 128 : (it + 1) * 128],
                            rhs=augs[jt],
                            start=(jt == 0),
                            stop=(jt == NT - 1),
                        )

            # -- normalize + single batched store --------------------------
            o_g = opool.tile([128, NT * H], FP32)
            rz = rzpool.tile([128, 8], FP32)
            for half in range(2):
                zs = pos[half][:].rearrange("p (s c) -> p s c", s=4, c=65)[:, :, H : H + 1]
                nc.vector.reciprocal(
                    rz[:, half * 4 : half * 4 + 4],
                    zs.rearrange("p s one -> p (s one)"),
                )
            for it in range(NT):
                half, r = it // 4, it % 4
                src = pos[half][:, r * 65 : r * 65 + H]
                dst = o_g[:, it * H : (it + 1) * H]
                sc = rz[:, it : it + 1]
                if it < EPI_S:
                    nc.scalar.activation(dst, src, AF.Copy, scale=sc)
                else:
                    nc.vector.tensor_scalar(dst, src, sc, None, OP.mult)
            og_dst = out[g].rearrange("(it p) c -> p it c", it=NT, p=128)
            nc.sync.dma_start(og_dst, o_g[:].rearrange("p (it c) -> p it c", it=NT))

        def per_rep_body():
            xts = {0: emit_dma(0)}
            stage_a_out = {}
            for g in range(N_PER + 1):
                if g + 1 <= N_PER - 1:
                    xts[g + 1] = emit_dma(g + 1)
                if g < N_PER:
                    stage_a_out[g] = emit_loop_a(g, xts.pop(g))
                if g >= 1:
                    emit_stage_b(g - 1, *stage_a_out.pop(g - 1))

        if hw_loop and reps > 1:
            with tc.For_i(0, reps):
                for _ in range(body_reps):
                    per_rep_body()
        else:
            for _ in range(reps):
                per_rep_body()

    nc.compile()
    return nc


_NC_CACHE = None

FEAT_NP_DTYPE = mybir.dt.np(BF16)


def _get_program():
    global _NC_CACHE
    if _NC_CACHE is None:
        _NC_CACHE = build_gat_program()
    return _NC_CACHE


def kernel(features: np.ndarray, W: np.ndarray, a: np.ndarray) -> np.ndarray:
    """Full-input entry point: features [32, 1024, 128], W [64, 128], a [1, 128]."""
    assert features.shape == (N_TOTAL, V, F)
    nc = _get_program()

    features = np.ascontiguousarray(features, dtype=np.float32).astype(FEAT_NP_DTYPE)
    W = np.ascontiguousarray(W, dtype=np.float32)
    a = np.ascontiguousarray(a, dtype=np.float32)

    in_maps = [
        {
            "features": features[c * N_PER : (c + 1) * N_PER],
            "W": W,
            "a": a,
        }
        for c in range(N_CORES)
    ]
    res = run_bass_kernel_spmd(nc, in_maps, core_ids=list(range(N_CORES)))
    outs = [res.results[c]["out"] for c in range(N_CORES)]
    return np.concatenate(outs, axis=0)


if __name__ == "__main__":
    prog = build_gat_program()
    print("program built ok")


# revision 8
# speedup vs baseline: 2.3991x; 1.3117x over previous
"""GAT layer kernel for Trainium2, data-parallel over 8 NeuronCores.

Problem (per graph): X [1024, 128] f32, W [64, 128], a [1, 128]
  h = X @ W.T                       [1024, 64]
  s_src = h @ a[:64], s_dst = h @ a[64:]
  e[i,j] = leaky_relu(s_src[i] + s_dst[j], 0.01)
  att = softmax_j(e); out = att @ h  [1024, 64]

32 graphs total -> 4 per core across 8 cores (inputs W/a replicated).

Per-core kernel strategy (v3):
  - Features are converted to bf16 on the host (they feed bf16 matmuls
    anyway) and X.T is produced by the DMA xbar-transpose engine
    (dma_start(..., transpose=True)) directly into SBUF as
    xt[f, v] = X[v, f].  This kills the 8 PE transposes AND the 8 wide
    PSUM->SBUF xtb copies per graph of v2, and halves input DMA bytes.
  - Attention built directly in TRANSPOSED layout PT[j, i] (the lhsT the
    accumulation matmul needs).  exp(lrelu(x)) = max(exp(x), exp(x/100));
    for |x| <~ 8 the second branch is exp(x/100) = 1 + x/100 + O(3e-3),
    and since it only wins where e < 0 (value ~1), dropping its
    i-dependence costs O(1%) on near-1 entries that largely cancels in
    the softmax ratio.  So:
        PT[j, i] = max(exp(s_src_i) * exp(s_dst_j),  1 + 0.01*s_dst_j)
                 = tensor_scalar(a_rep, scalar1=b_j, scalar2=tau_j,
                                 op0=mult, op1=max)
    ONE 4x-mode DVE op per 128x1024 tile (bf16, all-SBUF).
  - a_rep[m, i] = exp(s_src_i) replicated across partitions via a
    column-replicated weight vector (wsrc_rep): PSUM gets
    srep[m, i] = s_src_i on every partition with TWO matmuls per graph
    (rhs = xt halves, N=512 each), then ONE wide exp.
  - s_dst rides the h matmul (rhs = [w_dst | W.T], one xtb weight load
    for both outputs): 8 matmuls of N=65 -> two packed PSUM banks
    (4 x 65 cols each), copied to SBUF slots [sdst | h | 1] with TWO
    batched copies per graph; b/tau are built from the bf16 s_dst
    values with strided-tiny ScalarE ops.
  - A ones column follows h in each slot so PT.T @ [h | 1] produces both
    h' and the softmax normalizer Z in PSUM; epilogue multiplies by 1/Z
    (reciprocals batched 4 cols at a time).
  - Emission is stage-skewed: loop A (h/scores/exp) of graph g+1 is
    emitted before stage B (attention build + accumulate + epilogue) of
    graph g; X.T DMAs prefetch one graph ahead.  PSUM: mh 2 banks +
    srep 2 + poA 2 + poB 2 = 8 (exactly the budget).
  - HW notes: GpSimd cannot touch PSUM and its launch overhead is large,
    so it only gets the one-time ones-column memset.  PSUM->SBUF copies
    are split between ScalarE and DVE for engine balance (knobs below).
"""

import os
import sys

if "/opt/trn_rl_repo" not in sys.path:
    sys.path.insert(0, "/opt/trn_rl_repo")

from contextlib import ExitStack

import numpy as np

import concourse.bass as bass
import concourse.mybir as mybir
import concourse.tile as tile
from concourse import bacc
from concourse.bass_utils import run_bass_kernel_spmd
from concourse.masks import make_identity

# ---- hardcoded problem shapes -------------------------------------------
N_TOTAL = 32          # graphs
N_CORES = 8
N_PER = N_TOTAL // N_CORES   # 4 graphs per core
V = 1024              # nodes per graph
F = 128               # input features
H = 64                # hidden features
NT = V // 128         # 8 tiles of 128 nodes
SLOPE = 0.01          # leaky_relu negative slope
SLOT = 66             # aug slot: [sdst | h(64) | one]

FP32 = mybir.dt.float32
BF16 = mybir.dt.bfloat16
AF = mybir.ActivationFunctionType
OP = mybir.AluOpType

# engine-balance knobs
MH_S = int(os.environ.get("GAT_MH_S", "1"))    # mh combined copy on ScalarE (1) or DVE (0)
EPI_S = int(os.environ.get("GAT_EPI_S", "7"))  # epi scales on ScalarE (of 8)


def build_gat_program(reps: int = 1, hw_loop: bool = False, body_reps: int = 1):
    """Build the per-core Bass program (same program on all 8 cores)."""
    nc = bacc.Bacc("TRN2", target_bir_lowering=False, debug=False)

    feat_d = nc.dram_tensor("features", [N_PER, V, F], BF16, kind="ExternalInput")
    w_d = nc.dram_tensor("W", [H, F], FP32, kind="ExternalInput")
    a_d = nc.dram_tensor("a", [1, 2 * H], FP32, kind="ExternalInput")
    out_d = nc.dram_tensor("out", [N_PER, V, H], FP32, kind="ExternalOutput")

    feat = feat_d.ap()
    out = out_d.ap()

    with tile.TileContext(nc) as tc, ExitStack() as ctx:
        # ---- pools -------------------------------------------------------
        consts = ctx.enter_context(tc.tile_pool(name="consts", bufs=1))
        xtpool = ctx.enter_context(tc.tile_pool(name="xt", bufs=3))
        reppool = ctx.enter_context(tc.tile_pool(name="rep", bufs=2))
        btpool = ctx.enter_context(tc.tile_pool(name="bt", bufs=2))
        ppool = ctx.enter_context(tc.tile_pool(name="p", bufs=2 * NT))
        rzpool = ctx.enter_context(tc.tile_pool(name="rz", bufs=2))
        opool = ctx.enter_context(tc.tile_pool(name="o", bufs=2))

        # PSUM bank budget (8 total, 2KB per partition per bank):
        #   ps_mh  : [128, 1024] f32 -> 2 banks x1 buf = 2
        #     (4x65-col [sdst|h] tiles at cols 0:260 of each bank -> ONE
        #      batched PSUM->SBUF copy per graph)
        #   ps_srep: [128, 1024] f32 -> 2 banks x1 buf = 2
        #   ps_po  : [128, 1024] f32 -> 2 banks x2 bufs = 4
        #     (4x65-col h' tiles at cols 0:260 of each bank -> ONE batched
        #      reciprocal per graph)
        ps_mh = ctx.enter_context(tc.tile_pool(name="ps_mh", bufs=1, space="PSUM"))
        ps_srep = ctx.enter_context(tc.tile_pool(name="ps_srep", bufs=1, space="PSUM"))
        ps_po = ctx.enter_context(tc.tile_pool(name="ps_po", bufs=2, space="PSUM"))

        # ---- constants / weight prep ------------------------------------
        ident = consts.tile([128, 128], FP32)
        make_identity(nc, ident[:])

        a_sb = consts.tile([1, 2 * H], FP32)
        nc.sync.dma_start(a_sb[:], a_d.ap()[:])
        w_sb = consts.tile([H, F], FP32)
        nc.sync.dma_start(w_sb[:], w_d.ap()[:])

        # a halves -> f32 columns [H, 2] (via PE transpose of the row)
        asrc_ps = ps_mh.tile([H, 1], FP32, tag="mh")
        nc.tensor.transpose(asrc_ps[:], a_sb[0:1, 0:H], ident[0:1, 0:1])
        adst_ps = ps_mh.tile([H, 1], FP32, tag="mh")
        nc.tensor.transpose(adst_ps[:], a_sb[0:1, H : 2 * H], ident[0:1, 0:1])
        a2 = consts.tile([H, 2], FP32)
        nc.vector.tensor_copy(a2[:, 0:1], asrc_ps[:])
        nc.vector.tensor_copy(a2[:, 1:2], adst_ps[:])

        # w_src/w_dst = W.T @ a_halves : [F, 2] (fp32 one-time matmul)
        wcols_ps = ps_mh.tile([F, 2], FP32, tag="mh")
        nc.tensor.matmul(wcols_ps[:], lhsT=w_sb[:], rhs=a2[:], start=True, stop=True)
        # column-replicated w_src: wsrc_rep[f, m] = w_src[f] for all m
        wsrc_rep = consts.tile([F, 128], BF16)
        nc.scalar.copy(wsrc_rep[:], wcols_ps[:, 0:1].broadcast_to((F, 128)))

        # rhs_w = [w_dst | W.T] : [F, 1+H] bf16 -- the h matmul then yields
        # [s_dst | h] in one pass with one weight load of the xt slice
        wt_ps = ps_mh.tile([F, H], FP32, tag="mh")
        nc.tensor.transpose(wt_ps[:], w_sb[:], ident[0:H, 0:H])
        rhs_w = consts.tile([F, 1 + H], BF16)
        nc.vector.tensor_copy(rhs_w[:, 0:1], wcols_ps[:, 1:2])
        nc.vector.tensor_copy(rhs_w[:, 1 : 1 + H], wt_ps[:])

        # persistent aug slots [sdst(1) | h(64) | 1]: the batched
        # PSUM->SBUF copies fill cols 0:65; the ones column (65) is
        # written once here.  2-graph cycle x NT slots.
        augbig = consts.tile([128, 2 * NT * SLOT], BF16)
        nc.gpsimd.memset(
            augbig[:].rearrange("p (s c) -> p s c", s=2 * NT, c=SLOT)[
                :, :, SLOT - 1 : SLOT
            ],
            1.0,
        )

        # ---- per-graph pipeline -----------------------------------------
        def emit_dma(g):
            # whole-graph X.T via the DMA xbar transpose: [1024,128] bf16
            # in DRAM -> xt[f, v] in SBUF
            xt = xtpool.tile([128, V], BF16, name=f"xt_{g}", tag="xt")
            nc.sync.dma_start(xt[:], feat[g], transpose=True)
            return xt

        def emit_loop_a(g, xt):
            base = (g % 2) * NT

            def slot(j):
                s = base + j
                return augbig[:, s * SLOT : (s + 1) * SLOT]

            mhp = ps_mh.tile([128, 1024], FP32, name=f"mh{g}", tag="mh")
            srep_ps = ps_srep.tile([128, V], FP32, name="srep_ps")
            a_rep = reppool.tile([128, V], BF16, tag="a_rep")

            for jt in range(NT):
                hf, r = jt // 4, jt % 4
                # [s_dst | h] for node tile jt (one weight load of xt slice
                # serves both outputs), packed 4 tiles per PSUM bank
                nc.tensor.matmul(
                    mhp[:, hf * 512 + r * 65 : hf * 512 + (r + 1) * 65],
                    lhsT=xt[:, jt * 128 : (jt + 1) * 128],
                    rhs=rhs_w[:],
                    start=True,
                    stop=True,
                )
                if r == 3:
                    lo = hf * 4
                    # replicated s_src: one matmul per half-graph
                    nc.tensor.matmul(
                        srep_ps[:, lo * 128 : (lo + 4) * 128],
                        lhsT=wsrc_rep[:],
                        rhs=xt[:, lo * 128 : (lo + 4) * 128],
                        start=True,
                        stop=True,
                    )

            # ONE batched [sdst | h] copy for all 8 tiles
            dst = augbig[:, base * SLOT : (base + NT) * SLOT].rearrange(
                "p (hf r c) -> p hf r c", hf=2, r=4, c=SLOT)[:, :, :, 0:65]
            src = mhp[:].rearrange("p (hf x) -> p hf x", hf=2, x=512)[
                :, :, 0 : 4 * 65
            ].rearrange("p hf (r c) -> p hf r c", r=4, c=65)
            if MH_S:
                nc.scalar.copy(dst, src)
            else:
                nc.vector.tensor_copy(dst, src)

            # one wide exp for the whole a_rep
            nc.scalar.activation(a_rep[:], srep_ps[:], AF.Exp)

            # score scalars from the bf16 s_dst slot columns
            bt = btpool.tile([128, 16], FP32, tag="bt")
            sdin = augbig[:, base * SLOT : (base + NT) * SLOT].rearrange(
                "p (s c) -> p s c", s=NT, c=SLOT)[:, :, 0:1].rearrange(
                "p s one -> p (s one)")
            nc.scalar.activation(bt[:, 0:8], sdin, AF.Exp)
            nc.vector.tensor_scalar(bt[:, 8:16], sdin, SLOPE, 1.0,
                                    OP.mult, OP.add)
            augs = [slot(j)[:, 1:SLOT] for j in range(NT)]
            return augs, a_rep, bt

        def emit_stage_b(g, augs, a_rep, bt):
            if os.environ.get("GAT_SKIP_B"):
                return
            po = ps_po.tile([128, 1024], FP32, name=f"po_{g}", tag="po")
            p_ts = [ppool.tile([128, V], BF16, name=f"p{j}", tag="p_t") for j in range(NT)]
            for jt in range(NT):
                nc.vector.tensor_scalar(
                    p_ts[jt][:], a_rep[:], bt[:, jt : jt + 1],
                    bt[:, 8 + jt : 9 + jt], OP.mult, OP.max,
                )
            for half in range(2):
                for r in range(4):
                    it = half * 4 + r
                    for jt in range(NT):
                        nc.tensor.matmul(
                            po[:, half * 512 + r * 65 : half * 512 + (r + 1) * 65],
                            lhsT=p_ts[jt][:, it * 128 : (it + 1) * 128],
                            rhs=augs[jt],
                            start=(jt == 0),
                            stop=(jt == NT - 1),
                        )

            # -- normalize + single batched store --------------------------
            o_g = opool.tile([128, NT * H], FP32)
            rz = rzpool.tile([128, 8], FP32)
            zs = po[:].rearrange("p (hf x) -> p hf x", hf=2, x=512)[
                :, :, 0 : 4 * 65
            ].rearrange("p hf (r c) -> p hf r c", r=4, c=65)[
                :, :, :, H : H + 1
            ]
            rzv = rz[:, 0:8].rearrange("p (hf r one) -> p hf r one", hf=2, r=4, one=1)
            nc.vector.reciprocal(rzv, zs)
            for it in range(NT):
                half, r = it // 4, it % 4
                src = po[:, half * 512 + r * 65 : half * 512 + r * 65 + H]
                dst = o_g[:, it * H : (it + 1) * H]
                sc = rz[:, it : it + 1]
                if it < EPI_S:
                    nc.scalar.activation(dst, src, AF.Copy, scale=sc)
                else:
                    nc.vector.tensor_scalar(dst, src, sc, None, OP.mult)
            og_dst = out[g].rearrange("(it p) c -> p it c", it=NT, p=128)
            nc.sync.dma_start(og_dst, o_g[:].rearrange("p (it c) -> p it c", it=NT))

        def per_rep_body():
            xts = {0: emit_dma(0)}
            stage_a_out = {}
            for g in range(N_PER + 1):
                if g + 1 <= N_PER - 1:
                    xts[g + 1] = emit_dma(g + 1)
                if g < N_PER:
                    stage_a_out[g] = emit_loop_a(g, xts.pop(g))
                if g >= 1:
                    emit_stage_b(g - 1, *stage_a_out.pop(g - 1))

        if hw_loop and reps > 1:
            with tc.For_i(0, reps):
                for _ in range(body_reps):
                    per_rep_body()
        else:
            for _ in range(reps):
                per_rep_body()

    nc.compile()
    return nc


_NC_CACHE = None

FEAT_NP_DTYPE = mybir.dt.np(BF16)


def _get_program():
    global _NC_CACHE
    if _NC_CACHE is None:
        _NC_CACHE = build_gat_program()
    return _NC_CACHE


def kernel(features: np.ndarray, W: np.ndarray, a: np.ndarray) -> np.ndarray:
    """Full-input entry point: features [32, 1024, 128], W [64, 128], a [1, 128]."""
    assert features.shape == (N_TOTAL, V, F)
    nc = _get_program()

    features = np.ascontiguousarray(features, dtype=np.float32).astype(FEAT_NP_DTYPE)
    W = np.ascontiguousarray(W, dtype=np.float32)
    a = np.ascontiguousarray(a, dtype=np.float32)

    in_maps = [
        {
            "features": features[c * N_PER : (c + 1) * N_PER],
            "W": W,
            "a": a,
        }
        for c in range(N_CORES)
    ]
    res = run_bass_kernel_spmd(nc, in_maps, core_ids=list(range(N_CORES)))
    outs = [res.results[c]["out"] for c in range(N_CORES)]
    return np.concatenate(outs, axis=0)


if __name__ == "__main__":
    prog = build_gat_program()
    print("program built ok")
